# revision 4
# baseline (speedup 1.0000x reference)
"""Trainium2 Bass kernel for the 8-bit SNN barrel shifter.

Reference semantics (all inputs are exactly 0.0/1.0 f32):
    shift = S[:,0] + 2*S[:,1] + 4*S[:,2]
    out[:, i] = P[:, i - shift] if i >= shift else 0

Device strategy (pure data parallel over 8 cores, row-major layout):
  - host repacks P/S to uint8 (0/1 bits) and shards rows across cores
  - per core, the vector engine packs each row's 8 bits into an int16 via a
    shift+add Horner tree, packs the 3 shift bits the same way, applies one
    per-element logical_shift_left, then extracts the 8 output bit planes
  - bit planes are stored planar (uint8); host re-interleaves and casts back
    to f32
"""
import numpy as np

_N = 4194304
_CORES = 8
_NC = _N // _CORES          # rows per core
_PARTS = 128
_R = 512                    # rows per partition per tile
_T = _NC // (_PARTS * _R)   # tiles per core

_CACHE: dict = {}


def _build(rows_per_core: int, R: int):
    import concourse.tile as tile
    from concourse import bacc, mybir

    dt = mybir.dt
    Alu = mybir.AluOpType
    P = _PARTS
    T = rows_per_core // (P * R)
    assert T * P * R == rows_per_core

    nc = bacc.Bacc("TRN2", target_bir_lowering=False, debug=False)
    p8 = nc.dram_tensor("p8", (rows_per_core, 8), dt.uint8, kind="ExternalInput").ap()
    s8 = nc.dram_tensor("s8", (rows_per_core, 4), dt.uint8, kind="ExternalInput").ap()
    o8 = nc.dram_tensor("o8", (T, P, 8, R), dt.uint8, kind="ExternalOutput").ap()

    pr = p8.rearrange("(t p r) c -> t p r c", t=T, p=P, r=R)
    sr = s8.rearrange("(t p r) c -> t p r c", t=T, p=P, r=R)

    with tile.TileContext(nc) as tc:
        with tc.tile_pool(name="io", bufs=3) as io, tc.tile_pool(name="tmp", bufs=2) as tmp:
            for t in range(T):
                pt = io.tile([P, R, 8], dt.uint8, tag="p")
                st = io.tile([P, R, 4], dt.uint8, tag="s")
                nc.sync.dma_start(pt[:], pr[t])
                nc.sync.dma_start(st[:], sr[t])

                # pack P bits: Horner tree, v = sum_i b_i 2^i
                c1 = tmp.tile([P, R, 4], dt.uint8, tag="c1")
                nc.vector.scalar_tensor_tensor(
                    c1[:], pt[:, :, 1::2], 1, pt[:, :, 0::2],
                    op0=Alu.logical_shift_left, op1=Alu.bitwise_or)
                c2 = tmp.tile([P, R, 2], dt.uint8, tag="c2")
                nc.vector.scalar_tensor_tensor(
                    c2[:], c1[:, :, 1::2], 2, c1[:, :, 0::2],
                    op0=Alu.logical_shift_left, op1=Alu.bitwise_or)
                vi = tmp.tile([P, R], dt.uint8, tag="vi")
                nc.vector.scalar_tensor_tensor(
                    vi[:], c2[:, :, 1], 4, c2[:, :, 0],
                    op0=Alu.logical_shift_left, op1=Alu.bitwise_or)

                # pack S bits: t = s0 + 2*s1 + 4*s2 (Horner)
                a = tmp.tile([P, R], dt.uint8, tag="a")
                nc.vector.scalar_tensor_tensor(
                    a[:], st[:, :, 2], 1, st[:, :, 1],
                    op0=Alu.logical_shift_left, op1=Alu.bitwise_or)
                ti = tmp.tile([P, R], dt.uint8, tag="ti")
                nc.vector.scalar_tensor_tensor(
                    ti[:], a[:], 1, st[:, :, 0],
                    op0=Alu.logical_shift_left, op1=Alu.bitwise_or)

                # vs = vi << ti  (per-element shift)
                vs = tmp.tile([P, R], dt.uint8, tag="vs")
                nc.vector.tensor_tensor(vs[:], vi[:], ti[:], op=Alu.logical_shift_left)

                # unpack: 8 bit planes, planar layout
                ot = io.tile([P, 8, R], dt.uint8, tag="o")
                for i in range(8):
                    nc.vector.tensor_scalar(
                        ot[:, i, :], vs[:], i, 1,
                        op0=Alu.logical_shift_right, op1=Alu.bitwise_and)

                nc.sync.dma_start(o8[t], ot[:])
    nc.compile()
    _fix_bitwise_imms(nc, mybir)
    return nc


_BITWISE = None


def _fix_bitwise_imms(nc, mybir):
    """walrus requires integer immediates matching the src dtype on bitvec
    tensor_scalar ops; bass emits float32/int32 — rewrite them."""
    global _BITWISE
    Alu = mybir.AluOpType
    if _BITWISE is None:
        _BITWISE = {
            Alu.bitwise_and, Alu.bitwise_or, Alu.bitwise_xor, Alu.bitwise_not,
            Alu.logical_shift_left, Alu.logical_shift_right,
            Alu.arith_shift_left, Alu.arith_shift_right,
        }
    for f in nc.m.functions:
        for blk in f.blocks:
            for i in blk.instructions:
                if type(i).__name__ != "InstTensorScalarPtr":
                    continue
                ops = [getattr(i, "op0", None), getattr(i, "op1", None)]
                if not any(op in _BITWISE for op in ops if op is not None):
                    continue
                src_dt = i.ins[0].dtype
                for k in range(1, len(i.ins)):
                    iv = i.ins[k]
                    if isinstance(iv, mybir.ImmediateValue):
                        i.ins[k] = mybir.ImmediateValue(
                            dtype=src_dt, value=int(iv.value))


def _get_nc():
    key = (_NC, _R)
    if key not in _CACHE:
        _CACHE[key] = _build(*key)
    return _CACHE[key]


def kernel(P: np.ndarray, S: np.ndarray) -> np.ndarray:
    from concourse.bass_utils import run_bass_kernel_spmd

    nc = _get_nc()

    Pb = np.ascontiguousarray(P).astype(np.uint8)      # exact 0/1
    s8 = np.zeros((_N, 4), np.uint8)
    s8[:, :3] = np.ascontiguousarray(S).astype(np.uint8)

    in_maps = [
        {"p8": Pb[c * _NC:(c + 1) * _NC], "s8": s8[c * _NC:(c + 1) * _NC]}
        for c in range(_CORES)
    ]
    res = run_bass_kernel_spmd(nc, in_maps, core_ids=list(range(_CORES)))

    out = np.empty((_N, 8), np.float32)
    for c, r in enumerate(res.results):
        o = r["o8"].reshape(_T, _PARTS, 8, _R)
        rows = o.transpose(0, 1, 3, 2).reshape(_NC, 8)
        out[c * _NC:(c + 1) * _NC] = rows
    return out
